# revision 15
# baseline (speedup 1.0000x reference)
"""Multi-head graph attention (GAT-style message passing) on 8 Trainium2 cores.

Math (per head i, diag transform):
    h        = x * w[i]                      # [N, d]
    p_src    = h @ a[:d],  p_dst = h @ a[d:] # [N]
    s_e      = p_src[src_e] + p_dst[dst_e]   # per edge
    e_e      = exp(-leaky_relu(s_e, 0.2))
    out[i,n] = (sum_{e: src=n} e_e * h[dst_e]) / (sum_{e: src=n} e_e)

Key identities:
  - w[i] commutes with the segment sum, so the x[dst] gather is shared by all
    4 heads and w[i] is applied at the very end.
  - exp(-leaky_relu(s)) = min(exp(-s), exp(-0.2 s)) and s factors into
    p_src[src] + p_dst[dst], so the per-edge weight is
    min(A[src]*B[dst], C[src]*D[dst]) with A,C per source column and B,D
    carried by the gathered destination row.

Layout ("quarter-window" scheme):
  - Edges are sorted by src and partitioned across 8 cores by src range.
  - A super-window = up to 32 consecutive src nodes whose per-dst-quarter
    edge counts each fit in 128.  Each window emits 4 chunks (one per dst
    quarter, 128 edge slots each); dst indices within a chunk are local to
    that quarter (< 25000), so they fit the int16 indices of the gpsimd
    dma_gather custom op.  One dma_gather per (group, quarter) replaces the
    128-row indirect DMAs that dominated the old kernel (994ns+ each on the
    Pool engine).
  - A group = 8 windows = 32 chunks = 4096 edge slots.  The 4 chunks of a
    window accumulate into the same PSUM block (matmul start/stop), so
    per-window rowsums and aggregates are complete on device.
  - p_src per edge slot comes from a host-built transposed one-hot (m0T)
    matmul against the window's 32 column P-values (gathered per group from
    a per-core P table, int16-local again).
  - Output is written f16, [4, G, 128, 8*32] per core, host scatters rows.
"""

import os

import numpy as np

from concourse import bacc, bass, mybir
import concourse.tile as tile
from concourse.bass_utils import run_bass_kernel_spmd

LAST_RESULTS = []

F32 = mybir.dt.float32
F16 = mybir.dt.float16
I16 = mybir.dt.int16

N_CORES = 8
W = 32            # nodes per super-window (one-hot width)
NQ = 4            # dst quarters
EPC = 128         # edge slots per chunk (partition dim)
SPG = 8           # super-windows per group
CPG = SPG * NQ    # chunks per group
EPG = CPG * EPC   # edge slots per group (4096)
OPG = SPG * W     # output node slots per group (256)


# --------------------------------------------------------------------------
# host-side layout preprocessing
# --------------------------------------------------------------------------

def _prep_core(src_s, dst_s, n_lo, n_hi, qr):
    """Pack one core's src-sorted edges into quarter-window chunks.

    Returns per-core arrays (no group padding; caller pads to uniform G):
      n_win, wbases[n_win], spans[n_win],
      e_slot (slot index per edge within G*EPG), e_idx (int16 dst local id),
      e_loc (src offset within window).
    """
    npc = n_hi - n_lo
    lo = np.searchsorted(src_s, n_lo, side="left")
    hi = np.searchsorted(src_s, n_hi, side="left")
    s_loc = (src_s[lo:hi] - n_lo).astype(np.int64)
    d = dst_s[lo:hi].astype(np.int64)
    q = d // qr

    cnt = np.bincount(s_loc * NQ + q, minlength=npc * NQ).reshape(npc, NQ)
    assert cnt.max() <= EPC, "node quarter-degree exceeds one chunk"

    wbases = [0]
    cur = cnt[0].astype(np.int64).copy()
    wid = np.empty(npc, np.int32)
    wid[0] = 0
    for n in range(1, npc):
        c = cnt[n]
        if (n - wbases[-1] >= W) or np.any(cur + c > EPC):
            wbases.append(n)
            cur = c.astype(np.int64).copy()
        else:
            cur += c
        wid[n] = len(wbases) - 1
    wbases = np.asarray(wbases, np.int64)
    n_win = len(wbases)
    spans = np.empty(n_win, np.int64)
    spans[:-1] = wbases[1:] - wbases[:-1]
    spans[-1] = npc - wbases[-1]

    # chunk id per edge, then stable sort so each chunk's edges are contiguous
    e_w = wid[s_loc]
    e_ch = e_w.astype(np.int64) * NQ + q
    order = np.argsort(e_ch, kind="stable")
    e_ch = e_ch[order]
    e_d = d[order]
    e_q = q[order]
    e_loc = (s_loc - wbases[e_w])[order]

    n_ch = n_win * NQ
    ch_cnt = np.bincount(e_ch, minlength=n_ch)
    ch_start = np.concatenate([[0], np.cumsum(ch_cnt)[:-1]])
    rank = np.arange(len(e_ch)) - ch_start[e_ch]

    # slot within [G, CPG, EPC]: group g = w//SPG, kk = q*SPG + (w%SPG)
    w_of = e_ch // NQ
    q_of = e_ch % NQ
    g = w_of // SPG
    kk = q_of * SPG + (w_of % SPG)
    e_slot = (g * CPG + kk) * EPC + rank

    e_idx = (e_d - e_q * qr).astype(np.int16)
    return n_win, wbases, spans, e_slot, e_idx, e_loc.astype(np.float16)


def _wrap16(arr2d):
    """[n, 16*k] idx array -> dma_gather wrapped layout [16, k] per row set.

    arr2d: [rows, num_idxs]; returns [rows, 16, num_idxs//16] with
    out[r, p, f] = arr2d[r, f*16 + p].
    """
    r, n = arr2d.shape
    return arr2d.reshape(r, n // 16, 16).transpose(0, 2, 1)


def _prep_edges(src, dst, n_nodes, qr):
    npc = n_nodes // N_CORES
    order = np.argsort(src, kind="stable")
    src_s = src[order]
    dst_s = dst[order]

    cores = []
    for c in range(N_CORES):
        cores.append(_prep_core(src_s, dst_s, c * npc, (c + 1) * npc, qr))
    G = max((cr[0] + SPG - 1) // SPG for cr in cores)

    metas, locms, m0Ts, cmaps = [], [], [], []
    for c in range(N_CORES):
        n_win, wbases, spans, e_slot, e_idx, e_loc = cores[c]

        xidx = np.zeros(G * EPG, np.int16)
        xidx[e_slot] = e_idx
        locv = np.full(G * EPG, -1.0, np.float16)
        locv[e_slot] = e_loc
        locq = locv.reshape(G, CPG, EPC)

        # m0T one-hot [G, W, CPG*EPC]: m0T[g, w, kk*EPC+p] = (loc(kk,p)==w)
        oh = (locq.reshape(G, CPG, 1, EPC)
              == np.arange(W, dtype=np.float16).reshape(1, 1, W, 1)
              ).astype(np.float16)              # [G, CPG, W, EPC]
        m0T = oh.transpose(0, 2, 1, 3).reshape(G, W, CPG * EPC).copy()

        # device loc layout [G, 128, CPG]
        locm = locq.transpose(0, 2, 1).copy()

        # per-window columns: pcT gather idx (int16 local node id) + colmap
        s_all = np.arange(G * SPG)
        wb = np.zeros(G * SPG, np.int64)
        sp = np.zeros(G * SPG, np.int64)
        wb[:n_win] = wbases
        sp[:n_win] = spans
        ww = np.arange(W)
        colnode = wb[:, None] + ww[None, :]          # [G*SPG, W]
        valid = ww[None, :] < sp[:, None]
        cmap = np.where(valid, colnode + c * npc, -1).astype(np.int64)

        # meta int16 [G, 128, NQ*QC]: cols = 4x wrapped xidx
        qc = SPG * EPC // 16
        meta = np.zeros((G, 128, NQ * qc), np.int16)
        xw = _wrap16(xidx.reshape(G * NQ, SPG * EPC)).reshape(G, NQ, 16, qc)
        for qq in range(NQ):
            meta[:, 0:16, qq * qc:(qq + 1) * qc] = xw[:, qq]
        meta[:, 16:32, :] = meta[:, 0:16, :]

        metas.append(meta)
        locms.append(locm)
        m0Ts.append(m0T)
        # output colmap in (s, w) order = s*32 + w
        cmaps.append(cmap.reshape(G, OPG))
    return dict(metas=metas, locms=locms, m0Ts=m0Ts, cmaps=cmaps, G=G)


# --------------------------------------------------------------------------
# launch 1: P = x @ A   (distributed over node slabs, batched by 4 tiles)
# --------------------------------------------------------------------------

def _build_l1(nt4):
    """xt: [128, nt4*512] f16 (x-slab transposed), amat: [128, 8] f16
    -> pout: [nt4*4, 128, 8] f32"""
    nc = bacc.Bacc(None)
    xt = nc.declare_dram_parameter("xt", [128, nt4 * 512], F16, isOutput=False)
    amat = nc.declare_dram_parameter("amat", [128, 8], F16, isOutput=False)
    pout = nc.declare_dram_parameter("pout", [nt4 * 4, 128, 8], F32,
                                     isOutput=True)

    with tile.TileContext(nc) as tc:
        with (
            tc.tile_pool(name="sb", bufs=3) as sb,
            tc.tile_pool(name="cst", bufs=1) as cst,
            tc.tile_pool(name="ps", bufs=2, space="PSUM") as ps,
        ):
            a_sb = cst.tile([128, 8], F16)
            nc.sync.dma_start(out=a_sb[:], in_=amat[:, :])
            dummy_ps = ps.tile([1, 1], F32, tag="dummy")
            nc.tensor.matmul(out=dummy_ps[:], lhsT=a_sb[:1, :1], rhs=a_sb[:1, :1],
                             start=True, stop=True)
            for t in range(nt4):
                xt_sb = sb.tile([128, 4, 128], F16, tag="xt")
                nc.sync.dma_start(out=xt_sb[:],
                                  in_=xt[:, t * 512:(t + 1) * 512])
                pp = ps.tile([128, 32], F32)
                for j in range(4):
                    nc.tensor.matmul(out=pp[:, j * 8:(j + 1) * 8],
                                     lhsT=xt_sb[:, j, :], rhs=a_sb[:],
                                     start=True, stop=True)
                p_sb = sb.tile([128, 4, 8], F32, tag="p")
                nc.vector.tensor_copy(out=p_sb[:], in_=pp[:].rearrange(
                    "p (j e) -> p j e", j=4, e=8))
                nc.sync.dma_start(
                    out=pout[t * 4:(t + 1) * 4].rearrange("j p e -> p j e"),
                    in_=p_sb[:])
    nc.compile()
    return nc


# --------------------------------------------------------------------------
# launch 2: the main edge-parallel kernel
# --------------------------------------------------------------------------

def _build_l2(G, qr, npc):
    nc = bacc.Bacc(None)
    qtabs = [nc.declare_dram_parameter(f"q{i}", [qr, 256], F16, isOutput=False)
             for i in range(NQ)]
    pcolmp = nc.declare_dram_parameter("pcolm", [G, W, SPG * 4], F16,
                                       isOutput=False)
    meta = nc.declare_dram_parameter("meta", [G, 128, NQ * SPG * EPC // 16], I16, isOutput=False)
    locmp = nc.declare_dram_parameter("locm", [G, 128, CPG], F16,
                                      isOutput=False)
    m0Tp = nc.declare_dram_parameter("m0T", [G, W, CPG * EPC], F16,
                                     isOutput=False)
    iotac = nc.declare_dram_parameter("iotac", [128, W], F16, isOutput=False)
    selc = nc.declare_dram_parameter("selc", [4, 512], F16, isOutput=False)
    wcol = nc.declare_dram_parameter("wcol", [128, 4], F32, isOutput=False)
    out = nc.declare_dram_parameter("out", [4, G, 128, OPG], F16,
                                    isOutput=True)

    with tile.TileContext(nc) as tc:
        with (
            tc.tile_pool(name="cst", bufs=1) as cst,
            tc.tile_pool(name="idx", bufs=3) as idxp,
            tc.tile_pool(name="gat", bufs=2) as gat,
            tc.tile_pool(name="mm", bufs=2) as mm,
            tc.tile_pool(name="epi", bufs=2) as epi,
            tc.tile_pool(name="outp", bufs=3) as outp,
            tc.tile_pool(name="psl", bufs=1, space="PSUM") as pslp,
            tc.tile_pool(name="agg", bufs=2, space="PSUM") as aggpool,
            tc.tile_pool(name="rs", bufs=1, space="PSUM") as rspool,
            tc.tile_pool(name="bc", bufs=1, space="PSUM") as bcpool,
        ):
            iota_sb = cst.tile([128, W], F16)
            nc.sync.dma_start(out=iota_sb[:], in_=iotac[:, :])
            sel_sb = cst.tile([4, 512], F16)
            nc.sync.dma_start(out=sel_sb[:], in_=selc[:, :])
            w_sb = cst.tile([128, 4], F32)
            nc.sync.dma_start(out=w_sb[:], in_=wcol[:, :])

            for g in range(G):
                meta_sb = idxp.tile([128, NQ * SPG * EPC // 16], I16, tag="meta")
                nc.sync.dma_start(out=meta_sb[:], in_=meta[g, :, :])
                loc_sb = idxp.tile([128, CPG], F16, tag="loc")
                nc.sync.dma_start(out=loc_sb[:], in_=locmp[g, :, :])
                m0T_sb = idxp.tile([W, CPG * EPC], F16, tag="m0T")
                nc.sync.dma_start(out=m0T_sb[:], in_=m0Tp[g, :, :])

                # ---- gathers: one per dst quarter + one for column P rows
                xg = gat.tile([128, CPG, 256], F16, tag="xg")
                for q in range(NQ):
                    nc.gpsimd.dma_gather(
                        xg[:, q * SPG:(q + 1) * SPG, :],
                        qtabs[q][:, :],
                        meta_sb[:, q * (SPG * EPC // 16):
                                (q + 1) * (SPG * EPC // 16)],
                        SPG * EPC, SPG * EPC, 256)
                pcol_sb = idxp.tile([W, SPG * 4], F16, tag="pcol")
                nc.sync.dma_start(out=pcol_sb[:], in_=pcolmp[g, :, :])

                # ---- one-hot m0 [128, CPG, W]
                m0 = mm.tile([128, CPG, W], F16, tag="m0")
                nc.vector.tensor_tensor(
                    out=m0[:],
                    in0=loc_sb[:, :, None].broadcast_to([128, CPG, W]),
                    in1=iota_sb[:, None, :].broadcast_to([128, CPG, W]),
                    op=mybir.AluOpType.is_equal)

                # ---- p_src per edge slot: P_slot = m0T^T . pcols
                psl = pslp.tile([128, CPG * 4], F32, tag="psl")
                for kk in range(CPG):
                    s = kk % SPG
                    nc.tensor.matmul(
                        out=psl[:, kk * 4:(kk + 1) * 4],
                        lhsT=m0T_sb[:, kk * EPC:(kk + 1) * EPC],
                        rhs=pcol_sb[:, s * 4:(s + 1) * 4],
                        start=True, stop=True)

                # ---- per-edge factors on ACT
                asl = mm.tile([128, CPG, 4], F16, tag="asl")
                nc.scalar.activation(out=asl[:].rearrange("p c i -> p (c i)"),
                                     in_=psl[:],
                                     func=mybir.ActivationFunctionType.Exp,
                                     scale=-1.0)
                csl = mm.tile([128, CPG, 4], F16, tag="csl")
                nc.scalar.activation(out=csl[:].rearrange("p c i -> p (c i)"),
                                     in_=psl[:],
                                     func=mybir.ActivationFunctionType.Exp,
                                     scale=-0.2)
                b16 = mm.tile([128, CPG, 4], F16, tag="b16")
                nc.scalar.activation(out=b16[:], in_=xg[:, :, 132:136],
                                     func=mybir.ActivationFunctionType.Exp,
                                     scale=-1.0)
                d16 = mm.tile([128, CPG, 4], F16, tag="d16")
                nc.scalar.activation(out=d16[:], in_=xg[:, :, 132:136],
                                     func=mybir.ActivationFunctionType.Exp,
                                     scale=-0.2)

                # ---- e = min(A*B, C*D)
                ab = mm.tile([128, CPG, 4], F16, tag="ab")
                nc.vector.tensor_tensor(out=ab[:], in0=asl[:], in1=b16[:],
                                        op=mybir.AluOpType.mult)
                cd = mm.tile([128, CPG, 4], F16, tag="cd")
                nc.vector.tensor_tensor(out=cd[:], in0=csl[:], in1=d16[:],
                                        op=mybir.AluOpType.mult)
                e4 = mm.tile([128, CPG, 4], F16, tag="e4")
                nc.vector.tensor_tensor(out=e4[:], in0=ab[:], in1=cd[:],
                                        op=mybir.AluOpType.min)

                # ---- weighted one-hot
                mall = mm.tile([128, CPG, 4, W], F16, tag="mall")
                nc.vector.tensor_tensor(
                    out=mall[:],
                    in0=m0[:, :, None, :].broadcast_to([128, CPG, 4, W]),
                    in1=e4[:, :, :, None].broadcast_to([128, CPG, 4, W]),
                    op=mybir.AluOpType.mult)

                # ---- segment sums; 4 quarter-chunks accumulate per window
                aggp = aggpool.tile([128, SPG * 4 * W], F32, tag="agg")
                rsp = rspool.tile([4, SPG * W], F32, tag="rs")
                for s in range(SPG):
                    for q in range(NQ):
                        kk = q * SPG + s
                        nc.tensor.matmul(
                            out=aggp[:, s * 4 * W:(s + 1) * 4 * W],
                            lhsT=xg[:, kk, 0:128], rhs=mall[:, kk, :, :],
                            start=(q == 0), stop=(q == 3))
                    for q in range(NQ):
                        kk = q * SPG + s
                        nc.tensor.matmul(
                            out=rsp[:, s * W:(s + 1) * W],
                            lhsT=e4[:, kk, :], rhs=m0[:, kk, :],
                            start=(q == 0), stop=(q == 3))

                # ---- reciprocal of rowsums (clamp pad zeros)
                rsc = epi.tile([4, OPG], F32, tag="rsc")
                nc.vector.tensor_scalar(out=rsc[:], in0=rsp[:], scalar1=3e-5,
                                        scalar2=None, op0=mybir.AluOpType.max)
                rsi16 = epi.tile([4, OPG], F16, tag="rsi16")
                with nc.allow_low_precision(reason="attention rowsum recip"):
                    nc.vector.reciprocal(out=rsi16[:], in_=rsc[:])

                # ---- epilogue: out = w * agg * (1/rowsum), f16
                agg4 = aggp[:].rearrange("p (s i w) -> p s i w", s=SPG, i=4,
                                         w=W)
                for i in range(4):
                    bc = bcpool.tile([128, OPG], F32, tag="bc")
                    nc.tensor.matmul(out=bc[:],
                                     lhsT=sel_sb[:, i * 128:(i + 1) * 128],
                                     rhs=rsi16[:], start=True, stop=True)
                    rinv = epi.tile([128, OPG], F32, tag="rinv")
                    nc.scalar.activation(out=rinv[:], in_=bc[:],
                                         func=mybir.ActivationFunctionType.Copy)
                    oh = outp.tile([128, OPG], F16, tag="oh")
                    nc.vector.scalar_tensor_tensor(
                        out=oh[:].rearrange("p (s w) -> p s w", s=SPG, w=W),
                        in0=agg4[:, :, i, :],
                        scalar=w_sb[:, i:i + 1],
                        in1=rinv[:].rearrange("p (s w) -> p s w", s=SPG, w=W),
                        op0=mybir.AluOpType.mult, op1=mybir.AluOpType.mult)
                    nc.sync.dma_start(out=out[i, g, :, :], in_=oh[:])
    nc.compile()
    return nc


# --------------------------------------------------------------------------
# entry point
# --------------------------------------------------------------------------

def kernel(x, w, attn, edge, _n_cores=N_CORES):
    x = np.asarray(x, dtype=np.float32)
    w = np.asarray(w, dtype=np.float32)
    attn = np.asarray(attn, dtype=np.float32)
    edge = np.asarray(edge)

    n_nodes, d = x.shape
    n_heads = w.shape[0]
    assert d == 128 and n_heads == 4
    qr = n_nodes // NQ
    npc = n_nodes // N_CORES

    src = edge[0].astype(np.int64)
    dst = edge[1].astype(np.int64)

    # fold parameters: A[:, i] = w_i * a_src_i ; A[:, 4+i] = w_i * a_dst_i
    amat = np.zeros((128, 8), dtype=np.float32)
    for i in range(n_heads):
        amat[:, i] = w[i, 0, :] * attn[i, :d, 0]
        amat[:, 4 + i] = w[i, 0, :] * attn[i, d:, 0]

    # ---------------- launch 1: P = x @ A (node slabs)
    nt = (npc + 127) // 128
    nt4 = (nt + 3) // 4
    nc1 = _build_l1(nt4)
    amat16 = amat.astype(np.float16)
    in_maps1 = []
    for c in range(N_CORES):
        sl = x[c * npc:(c + 1) * npc]
        if sl.shape[0] < nt4 * 512:
            sl = np.concatenate(
                [sl, np.zeros((nt4 * 512 - sl.shape[0], d), np.float32)])
        in_maps1.append({"xt": np.ascontiguousarray(sl.T).astype(np.float16),
                         "amat": amat16})
    trace = bool(int(os.environ.get("GAT_TRACE", "0")))
    tkw = dict(trace=True, trace_cores=list(range(N_CORES))) if trace else {}

    def _run(nc, maps):
        try:
            return run_bass_kernel_spmd(nc, maps, list(range(N_CORES)), **tkw)
        except Exception:
            if not tkw:
                raise
            return run_bass_kernel_spmd(nc, maps, list(range(N_CORES)))

    r1 = _run(nc1, in_maps1)
    ptab = np.concatenate(
        [r1.results[c]["pout"].reshape(-1, 8)[:npc] for c in range(N_CORES)],
        axis=0)

    # ---------------- host layout prep
    prep = _prep_edges(src, dst, n_nodes, qr)
    G = prep["G"]

    # ---------------- launch 2
    nc2 = _build_l2(G, qr, npc)
    t512 = np.zeros((n_nodes, 256), dtype=np.float16)
    t512[:, 0:128] = x.astype(np.float16)
    t512[:, 128:136] = ptab.astype(np.float16)
    qtabs = [np.ascontiguousarray(t512[i * qr:(i + 1) * qr])
             for i in range(NQ)]
    iota_c = np.broadcast_to(np.arange(W, dtype=np.float16), (128, W)).copy()
    sel_c = np.zeros((4, 512), dtype=np.float16)
    for i in range(4):
        sel_c[i, i * 128:(i + 1) * 128] = 1.0
    wcol = np.ascontiguousarray(w[:, 0, :].T)  # [128, 4]
    in_maps2 = []
    for c in range(N_CORES):
        cmap = prep["cmaps"][c].reshape(G, SPG, W)
        pc4 = ptab[np.maximum(cmap, 0), 0:4].astype(np.float16)
        pc4[cmap < 0] = 0
        pcolm = np.ascontiguousarray(
            pc4.transpose(0, 2, 1, 3).reshape(G, W, SPG * 4))
        m = {"pcolm": pcolm,
             "meta": prep["metas"][c], "locm": prep["locms"][c],
             "m0T": prep["m0Ts"][c],
             "iotac": iota_c, "selc": sel_c, "wcol": wcol}
        for i in range(NQ):
            m[f"q{i}"] = qtabs[i]
        in_maps2.append(m)
    r2 = _run(nc2, in_maps2)
    LAST_RESULTS.clear()
    LAST_RESULTS.extend([r1, r2])

    # ---------------- unshard: scatter window columns to node rows
    out_full = np.zeros((n_heads, n_nodes, d), dtype=np.float32)
    for c in range(N_CORES):
        slab = r2.results[c]["out"]      # [4, G, 128, OPG] f16
        cm = prep["cmaps"][c].reshape(-1)
        arr = slab.transpose(0, 1, 3, 2).reshape(n_heads, G * OPG, d)
        valid = cm >= 0
        out_full[:, cm[valid], :] = arr[:, valid, :].astype(np.float32)
    return out_full


if __name__ == "__main__":
    pass


# revision 16
# speedup vs baseline: 1.0430x; 1.0430x over previous
"""Multi-head graph attention (GAT-style message passing) on 8 Trainium2 cores.

Math (per head i, diag transform):
    h        = x * w[i]                      # [N, d]
    p_src    = h @ a[:d],  p_dst = h @ a[d:] # [N]
    s_e      = p_src[src_e] + p_dst[dst_e]   # per edge
    e_e      = exp(-leaky_relu(s_e, 0.2))
    out[i,n] = (sum_{e: src=n} e_e * h[dst_e]) / (sum_{e: src=n} e_e)

Key identities:
  - w[i] commutes with the segment sum, so the x[dst] gather is shared by all
    4 heads and w[i] is applied at the very end.
  - exp(-leaky_relu(s)) = min(exp(-s), exp(-0.2 s)) and s factors into
    p_src[src] + p_dst[dst], so the per-edge weight is
    min(A[src]*B[dst], C[src]*D[dst]) with A,C per source column and B,D
    carried by the gathered destination row.

Layout ("quarter-window" scheme):
  - Edges are sorted by src and partitioned across 8 cores by src range.
  - A super-window = up to 32 consecutive src nodes whose per-dst-quarter
    edge counts each fit in 128.  Each window emits 4 chunks (one per dst
    quarter, 128 edge slots each); dst indices within a chunk are local to
    that quarter (< 25000), so they fit the int16 indices of the gpsimd
    dma_gather custom op.  One dma_gather per (group, quarter) replaces the
    128-row indirect DMAs that dominated the old kernel (994ns+ each on the
    Pool engine).
  - A group = 8 windows = 32 chunks = 4096 edge slots.  The 4 chunks of a
    window accumulate into the same PSUM block (matmul start/stop), so
    per-window rowsums and aggregates are complete on device.
  - p_src per edge slot comes from a host-built transposed one-hot (m0T)
    matmul against the window's 32 column P-values (gathered per group from
    a per-core P table, int16-local again).
  - Output is written f16, [4, G, 128, 8*32] per core, host scatters rows.
"""

import os

import numpy as np

from concourse import bacc, bass, mybir
import concourse.tile as tile
from concourse.bass_utils import run_bass_kernel_spmd

LAST_RESULTS = []

F32 = mybir.dt.float32
F16 = mybir.dt.float16
I16 = mybir.dt.int16

N_CORES = 8
W = 32            # nodes per super-window (one-hot width)
NQ = 4            # dst quarters
EPC = 128         # edge slots per chunk (partition dim)
SPG = 8           # super-windows per group
CPG = SPG * NQ    # chunks per group
EPG = CPG * EPC   # edge slots per group (4096)
OPG = SPG * W     # output node slots per group (256)


# --------------------------------------------------------------------------
# host-side layout preprocessing
# --------------------------------------------------------------------------

def _prep_core(src_s, dst_s, n_lo, n_hi, qr):
    """Pack one core's src-sorted edges into quarter-window chunks.

    Returns per-core arrays (no group padding; caller pads to uniform G):
      n_win, wbases[n_win], spans[n_win],
      e_slot (slot index per edge within G*EPG), e_idx (int16 dst local id),
      e_loc (src offset within window).
    """
    npc = n_hi - n_lo
    lo = np.searchsorted(src_s, n_lo, side="left")
    hi = np.searchsorted(src_s, n_hi, side="left")
    s_loc = (src_s[lo:hi] - n_lo).astype(np.int64)
    d = dst_s[lo:hi].astype(np.int64)
    q = d // qr

    cnt = np.bincount(s_loc * NQ + q, minlength=npc * NQ).reshape(npc, NQ)
    assert cnt.max() <= EPC, "node quarter-degree exceeds one chunk"

    wbases = [0]
    cur = cnt[0].astype(np.int64).copy()
    wid = np.empty(npc, np.int32)
    wid[0] = 0
    for n in range(1, npc):
        c = cnt[n]
        if (n - wbases[-1] >= W) or np.any(cur + c > EPC):
            wbases.append(n)
            cur = c.astype(np.int64).copy()
        else:
            cur += c
        wid[n] = len(wbases) - 1
    wbases = np.asarray(wbases, np.int64)
    n_win = len(wbases)
    spans = np.empty(n_win, np.int64)
    spans[:-1] = wbases[1:] - wbases[:-1]
    spans[-1] = npc - wbases[-1]

    # chunk id per edge, then stable sort so each chunk's edges are contiguous
    e_w = wid[s_loc]
    e_ch = e_w.astype(np.int64) * NQ + q
    order = np.argsort(e_ch, kind="stable")
    e_ch = e_ch[order]
    e_d = d[order]
    e_q = q[order]
    e_loc = (s_loc - wbases[e_w])[order]

    n_ch = n_win * NQ
    ch_cnt = np.bincount(e_ch, minlength=n_ch)
    ch_start = np.concatenate([[0], np.cumsum(ch_cnt)[:-1]])
    rank = np.arange(len(e_ch)) - ch_start[e_ch]

    # slot within [G, CPG, EPC]: group g = w//SPG, kk = q*SPG + (w%SPG)
    w_of = e_ch // NQ
    q_of = e_ch % NQ
    g = w_of // SPG
    kk = q_of * SPG + (w_of % SPG)
    e_slot = (g * CPG + kk) * EPC + rank

    e_idx = (e_d - e_q * qr).astype(np.int16)
    return n_win, wbases, spans, e_slot, e_idx, e_loc.astype(np.float16)


def _wrap16(arr2d):
    """[n, 16*k] idx array -> dma_gather wrapped layout [16, k] per row set.

    arr2d: [rows, num_idxs]; returns [rows, 16, num_idxs//16] with
    out[r, p, f] = arr2d[r, f*16 + p].
    """
    r, n = arr2d.shape
    return arr2d.reshape(r, n // 16, 16).transpose(0, 2, 1)


def _prep_edges(src, dst, n_nodes, qr):
    npc = n_nodes // N_CORES
    order = np.argsort(src, kind="stable")
    src_s = src[order]
    dst_s = dst[order]

    cores = []
    for c in range(N_CORES):
        cores.append(_prep_core(src_s, dst_s, c * npc, (c + 1) * npc, qr))
    G = max((cr[0] + SPG - 1) // SPG for cr in cores)

    metas, locms, m0Ts, cmaps, dstgs = [], [], [], [], []
    for c in range(N_CORES):
        n_win, wbases, spans, e_slot, e_idx, e_loc = cores[c]

        xidx = np.zeros(G * EPG, np.int16)
        xidx[e_slot] = e_idx
        qof = (np.arange(G * EPG) // EPC % CPG) // SPG
        dstg = xidx.astype(np.int64) + qof * qr
        locv = np.full(G * EPG, -1.0, np.float16)
        locv[e_slot] = e_loc
        locq = locv.reshape(G, CPG, EPC)

        # m0T one-hot [G, W, CPG*EPC]: m0T[g, w, kk*EPC+p] = (loc(kk,p)==w)
        oh = (locq.reshape(G, CPG, 1, EPC)
              == np.arange(W, dtype=np.float16).reshape(1, 1, W, 1)
              ).astype(np.float16)              # [G, CPG, W, EPC]
        m0T = oh.transpose(0, 2, 1, 3).reshape(G, W, CPG * EPC).copy()

        # device loc layout [G, 128, CPG]
        locm = locq.transpose(0, 2, 1).copy()

        # per-window columns: pcT gather idx (int16 local node id) + colmap
        s_all = np.arange(G * SPG)
        wb = np.zeros(G * SPG, np.int64)
        sp = np.zeros(G * SPG, np.int64)
        wb[:n_win] = wbases
        sp[:n_win] = spans
        ww = np.arange(W)
        colnode = wb[:, None] + ww[None, :]          # [G*SPG, W]
        valid = ww[None, :] < sp[:, None]
        cmap = np.where(valid, colnode + c * npc, -1).astype(np.int64)

        # meta int16 [G, 128, NQ*QC]: cols = 4x wrapped xidx
        qc = SPG * EPC // 16
        meta = np.zeros((G, 128, NQ * qc), np.int16)
        xw = _wrap16(xidx.reshape(G * NQ, SPG * EPC)).reshape(G, NQ, 16, qc)
        for qq in range(NQ):
            meta[:, 0:16, qq * qc:(qq + 1) * qc] = xw[:, qq]
        meta[:, 16:32, :] = meta[:, 0:16, :]

        metas.append(meta)
        locms.append(locm)
        m0Ts.append(m0T)
        # output colmap in (s, w) order = s*32 + w
        cmaps.append(cmap.reshape(G, OPG))
        dstgs.append(dstg.reshape(G, CPG, EPC))
    return dict(metas=metas, locms=locms, m0Ts=m0Ts, cmaps=cmaps,
                dstgs=dstgs, G=G)


# --------------------------------------------------------------------------
# launch 1: P = x @ A   (distributed over node slabs, batched by 4 tiles)
# --------------------------------------------------------------------------

def _build_l1(nt4):
    """xt: [128, nt4*512] f16 (x-slab transposed), amat: [128, 8] f16
    -> pout: [nt4*4, 128, 8] f32"""
    nc = bacc.Bacc(None)
    xt = nc.declare_dram_parameter("xt", [128, nt4 * 512], F16, isOutput=False)
    amat = nc.declare_dram_parameter("amat", [128, 8], F16, isOutput=False)
    pout = nc.declare_dram_parameter("pout", [nt4 * 4, 128, 8], F32,
                                     isOutput=True)

    with tile.TileContext(nc) as tc:
        with (
            tc.tile_pool(name="sb", bufs=3) as sb,
            tc.tile_pool(name="cst", bufs=1) as cst,
            tc.tile_pool(name="ps", bufs=2, space="PSUM") as ps,
        ):
            a_sb = cst.tile([128, 8], F16)
            nc.sync.dma_start(out=a_sb[:], in_=amat[:, :])
            dummy_ps = ps.tile([1, 1], F32, tag="dummy")
            nc.tensor.matmul(out=dummy_ps[:], lhsT=a_sb[:1, :1], rhs=a_sb[:1, :1],
                             start=True, stop=True)
            for t in range(nt4):
                xt_sb = sb.tile([128, 4, 128], F16, tag="xt")
                nc.sync.dma_start(out=xt_sb[:],
                                  in_=xt[:, t * 512:(t + 1) * 512])
                pp = ps.tile([128, 32], F32)
                for j in range(4):
                    nc.tensor.matmul(out=pp[:, j * 8:(j + 1) * 8],
                                     lhsT=xt_sb[:, j, :], rhs=a_sb[:],
                                     start=True, stop=True)
                p_sb = sb.tile([128, 4, 8], F32, tag="p")
                nc.vector.tensor_copy(out=p_sb[:], in_=pp[:].rearrange(
                    "p (j e) -> p j e", j=4, e=8))
                nc.sync.dma_start(
                    out=pout[t * 4:(t + 1) * 4].rearrange("j p e -> p j e"),
                    in_=p_sb[:])
    nc.compile()
    return nc


# --------------------------------------------------------------------------
# launch 2: the main edge-parallel kernel
# --------------------------------------------------------------------------

def _build_l2(G, qr, npc):
    nc = bacc.Bacc(None)
    qtabs = [nc.declare_dram_parameter(f"q{i}", [qr, 256], F16, isOutput=False)
             for i in range(NQ)]
    pcolmp = nc.declare_dram_parameter("pcolm", [G, W, SPG * 4], F16,
                                       isOutput=False)
    pdmp = nc.declare_dram_parameter("pdm", [G, 128, CPG * 4], F16,
                                     isOutput=False)
    meta = nc.declare_dram_parameter("meta", [G, 128, NQ * SPG * EPC // 16], I16, isOutput=False)
    locmp = nc.declare_dram_parameter("locm", [G, 128, CPG], F16,
                                      isOutput=False)
    m0Tp = nc.declare_dram_parameter("m0T", [G, W, CPG * EPC], F16,
                                     isOutput=False)
    iotac = nc.declare_dram_parameter("iotac", [128, W], F16, isOutput=False)
    selc = nc.declare_dram_parameter("selc", [4, 512], F16, isOutput=False)
    wcol = nc.declare_dram_parameter("wcol", [128, 4], F32, isOutput=False)
    out = nc.declare_dram_parameter("out", [4, G, 128, OPG], F16,
                                    isOutput=True)

    with tile.TileContext(nc) as tc:
        with (
            tc.tile_pool(name="cst", bufs=1) as cst,
            tc.tile_pool(name="idx", bufs=3) as idxp,
            tc.tile_pool(name="gat", bufs=2) as gat,
            tc.tile_pool(name="mm", bufs=2) as mm,
            tc.tile_pool(name="epi", bufs=2) as epi,
            tc.tile_pool(name="outp", bufs=3) as outp,
            tc.tile_pool(name="psl", bufs=1, space="PSUM") as pslp,
            tc.tile_pool(name="agg", bufs=2, space="PSUM") as aggpool,
            tc.tile_pool(name="rs", bufs=1, space="PSUM") as rspool,
            tc.tile_pool(name="bc", bufs=1, space="PSUM") as bcpool,
        ):
            iota_sb = cst.tile([128, W], F16)
            nc.sync.dma_start(out=iota_sb[:], in_=iotac[:, :])
            sel_sb = cst.tile([4, 512], F16)
            nc.sync.dma_start(out=sel_sb[:], in_=selc[:, :])
            w_sb = cst.tile([128, 4], F32)
            nc.sync.dma_start(out=w_sb[:], in_=wcol[:, :])

            for g in range(G):
                meta_sb = idxp.tile([128, NQ * SPG * EPC // 16], I16, tag="meta")
                nc.sync.dma_start(out=meta_sb[:], in_=meta[g, :, :])
                loc_sb = idxp.tile([128, CPG], F16, tag="loc")
                nc.sync.dma_start(out=loc_sb[:], in_=locmp[g, :, :])
                m0T_sb = idxp.tile([W, CPG * EPC], F16, tag="m0T")
                nc.sync.dma_start(out=m0T_sb[:], in_=m0Tp[g, :, :])

                # ---- gathers: one per dst quarter + one for column P rows
                xg = gat.tile([128, CPG, 128], F16, tag="xg")
                for q in range(NQ):
                    nc.gpsimd.dma_gather(
                        xg[:, q * SPG:(q + 1) * SPG, :],
                        qtabs[q][:, 0:128],
                        meta_sb[:, q * (SPG * EPC // 16):
                                (q + 1) * (SPG * EPC // 16)],
                        SPG * EPC, SPG * EPC, 128, elem_step=256)
                pd_sb = idxp.tile([128, CPG, 4], F16, tag="pd")
                nc.sync.dma_start(
                    out=pd_sb[:].rearrange("p c i -> p (c i)"),
                    in_=pdmp[g, :, :])
                pcol_sb = idxp.tile([W, SPG * 4], F16, tag="pcol")
                nc.sync.dma_start(out=pcol_sb[:], in_=pcolmp[g, :, :])

                # ---- one-hot m0 [128, CPG, W]
                m0 = mm.tile([128, CPG, W], F16, tag="m0")
                nc.vector.tensor_tensor(
                    out=m0[:],
                    in0=loc_sb[:, :, None].broadcast_to([128, CPG, W]),
                    in1=iota_sb[:, None, :].broadcast_to([128, CPG, W]),
                    op=mybir.AluOpType.is_equal)

                # ---- p_src per edge slot: P_slot = m0T^T . pcols
                psl = pslp.tile([128, CPG * 4], F32, tag="psl")
                for kk in range(CPG):
                    s = kk % SPG
                    nc.tensor.matmul(
                        out=psl[:, kk * 4:(kk + 1) * 4],
                        lhsT=m0T_sb[:, kk * EPC:(kk + 1) * EPC],
                        rhs=pcol_sb[:, s * 4:(s + 1) * 4],
                        start=True, stop=True)

                # ---- per-edge factors on ACT
                asl = mm.tile([128, CPG, 4], F16, tag="asl")
                nc.scalar.activation(out=asl[:].rearrange("p c i -> p (c i)"),
                                     in_=psl[:],
                                     func=mybir.ActivationFunctionType.Exp,
                                     scale=-1.0)
                csl = mm.tile([128, CPG, 4], F16, tag="csl")
                nc.scalar.activation(out=csl[:].rearrange("p c i -> p (c i)"),
                                     in_=psl[:],
                                     func=mybir.ActivationFunctionType.Exp,
                                     scale=-0.2)
                b16 = mm.tile([128, CPG, 4], F16, tag="b16")
                nc.scalar.activation(out=b16[:], in_=pd_sb[:],
                                     func=mybir.ActivationFunctionType.Exp,
                                     scale=-1.0)
                d16 = mm.tile([128, CPG, 4], F16, tag="d16")
                nc.scalar.activation(out=d16[:], in_=pd_sb[:],
                                     func=mybir.ActivationFunctionType.Exp,
                                     scale=-0.2)

                # ---- e = min(A*B, C*D)
                ab = mm.tile([128, CPG, 4], F16, tag="ab")
                nc.vector.tensor_tensor(out=ab[:], in0=asl[:], in1=b16[:],
                                        op=mybir.AluOpType.mult)
                cd = mm.tile([128, CPG, 4], F16, tag="cd")
                nc.vector.tensor_tensor(out=cd[:], in0=csl[:], in1=d16[:],
                                        op=mybir.AluOpType.mult)
                e4 = mm.tile([128, CPG, 4], F16, tag="e4")
                nc.vector.tensor_tensor(out=e4[:], in0=ab[:], in1=cd[:],
                                        op=mybir.AluOpType.min)

                # ---- weighted one-hot
                mall = mm.tile([128, CPG, 4, W], F16, tag="mall")
                nc.vector.tensor_tensor(
                    out=mall[:],
                    in0=m0[:, :, None, :].broadcast_to([128, CPG, 4, W]),
                    in1=e4[:, :, :, None].broadcast_to([128, CPG, 4, W]),
                    op=mybir.AluOpType.mult)

                # ---- segment sums; 4 quarter-chunks accumulate per window
                aggp = aggpool.tile([128, SPG * 4 * W], F32, tag="agg")
                rsp = rspool.tile([4, SPG * W], F32, tag="rs")
                for s in range(SPG):
                    for q in range(NQ):
                        kk = q * SPG + s
                        nc.tensor.matmul(
                            out=aggp[:, s * 4 * W:(s + 1) * 4 * W],
                            lhsT=xg[:, kk, :], rhs=mall[:, kk, :, :],
                            start=(q == 0), stop=(q == 3))
                    for q in range(NQ):
                        kk = q * SPG + s
                        nc.tensor.matmul(
                            out=rsp[:, s * W:(s + 1) * W],
                            lhsT=e4[:, kk, :], rhs=m0[:, kk, :],
                            start=(q == 0), stop=(q == 3))

                # ---- reciprocal of rowsums (clamp pad zeros)
                rsc = epi.tile([4, OPG], F32, tag="rsc")
                nc.vector.tensor_scalar(out=rsc[:], in0=rsp[:], scalar1=3e-5,
                                        scalar2=None, op0=mybir.AluOpType.max)
                rsi16 = epi.tile([4, OPG], F16, tag="rsi16")
                with nc.allow_low_precision(reason="attention rowsum recip"):
                    nc.vector.reciprocal(out=rsi16[:], in_=rsc[:])

                # ---- epilogue: out = w * agg * (1/rowsum), f16
                agg4 = aggp[:].rearrange("p (s i w) -> p s i w", s=SPG, i=4,
                                         w=W)
                for i in range(4):
                    bc = bcpool.tile([128, OPG], F32, tag="bc")
                    nc.tensor.matmul(out=bc[:],
                                     lhsT=sel_sb[:, i * 128:(i + 1) * 128],
                                     rhs=rsi16[:], start=True, stop=True)
                    rinv = epi.tile([128, OPG], F32, tag="rinv")
                    nc.scalar.activation(out=rinv[:], in_=bc[:],
                                         func=mybir.ActivationFunctionType.Copy)
                    oh = outp.tile([128, OPG], F16, tag="oh")
                    nc.vector.scalar_tensor_tensor(
                        out=oh[:].rearrange("p (s w) -> p s w", s=SPG, w=W),
                        in0=agg4[:, :, i, :],
                        scalar=w_sb[:, i:i + 1],
                        in1=rinv[:].rearrange("p (s w) -> p s w", s=SPG, w=W),
                        op0=mybir.AluOpType.mult, op1=mybir.AluOpType.mult)
                    nc.sync.dma_start(out=out[i, g, :, :], in_=oh[:])
    nc.compile()
    return nc


# --------------------------------------------------------------------------
# entry point
# --------------------------------------------------------------------------

def kernel(x, w, attn, edge, _n_cores=N_CORES):
    x = np.asarray(x, dtype=np.float32)
    w = np.asarray(w, dtype=np.float32)
    attn = np.asarray(attn, dtype=np.float32)
    edge = np.asarray(edge)

    n_nodes, d = x.shape
    n_heads = w.shape[0]
    assert d == 128 and n_heads == 4
    qr = n_nodes // NQ
    npc = n_nodes // N_CORES

    src = edge[0].astype(np.int64)
    dst = edge[1].astype(np.int64)

    # fold parameters: A[:, i] = w_i * a_src_i ; A[:, 4+i] = w_i * a_dst_i
    amat = np.zeros((128, 8), dtype=np.float32)
    for i in range(n_heads):
        amat[:, i] = w[i, 0, :] * attn[i, :d, 0]
        amat[:, 4 + i] = w[i, 0, :] * attn[i, d:, 0]

    # ---------------- launch 1: P = x @ A (node slabs)
    nt = (npc + 127) // 128
    nt4 = (nt + 3) // 4
    nc1 = _build_l1(nt4)
    amat16 = amat.astype(np.float16)
    in_maps1 = []
    for c in range(N_CORES):
        sl = x[c * npc:(c + 1) * npc]
        if sl.shape[0] < nt4 * 512:
            sl = np.concatenate(
                [sl, np.zeros((nt4 * 512 - sl.shape[0], d), np.float32)])
        in_maps1.append({"xt": np.ascontiguousarray(sl.T).astype(np.float16),
                         "amat": amat16})
    trace = bool(int(os.environ.get("GAT_TRACE", "0")))
    tkw = dict(trace=True, trace_cores=list(range(N_CORES))) if trace else {}

    def _run(nc, maps):
        try:
            return run_bass_kernel_spmd(nc, maps, list(range(N_CORES)), **tkw)
        except Exception:
            if not tkw:
                raise
            return run_bass_kernel_spmd(nc, maps, list(range(N_CORES)))

    r1 = _run(nc1, in_maps1)
    ptab = np.concatenate(
        [r1.results[c]["pout"].reshape(-1, 8)[:npc] for c in range(N_CORES)],
        axis=0)

    # ---------------- host layout prep
    prep = _prep_edges(src, dst, n_nodes, qr)
    G = prep["G"]

    # ---------------- launch 2
    nc2 = _build_l2(G, qr, npc)
    t512 = np.zeros((n_nodes, 256), dtype=np.float16)
    t512[:, 0:128] = x.astype(np.float16)
    t512[:, 128:136] = ptab.astype(np.float16)
    qtabs = [np.ascontiguousarray(t512[i * qr:(i + 1) * qr])
             for i in range(NQ)]
    iota_c = np.broadcast_to(np.arange(W, dtype=np.float16), (128, W)).copy()
    sel_c = np.zeros((4, 512), dtype=np.float16)
    for i in range(4):
        sel_c[i, i * 128:(i + 1) * 128] = 1.0
    wcol = np.ascontiguousarray(w[:, 0, :].T)  # [128, 4]
    in_maps2 = []
    for c in range(N_CORES):
        cmap = prep["cmaps"][c].reshape(G, SPG, W)
        pc4 = ptab[np.maximum(cmap, 0), 0:4].astype(np.float16)
        pc4[cmap < 0] = 0
        pcolm = np.ascontiguousarray(
            pc4.transpose(0, 2, 1, 3).reshape(G, W, SPG * 4))
        dstg = prep["dstgs"][c]                      # [G, CPG, EPC]
        pdm = np.ascontiguousarray(
            ptab[dstg.transpose(0, 2, 1), 4:8].astype(np.float16)
            .reshape(G, 128, CPG * 4))
        m = {"pcolm": pcolm, "pdm": pdm,
             "meta": prep["metas"][c], "locm": prep["locms"][c],
             "m0T": prep["m0Ts"][c],
             "iotac": iota_c, "selc": sel_c, "wcol": wcol}
        for i in range(NQ):
            m[f"q{i}"] = qtabs[i]
        in_maps2.append(m)
    r2 = _run(nc2, in_maps2)
    LAST_RESULTS.clear()
    LAST_RESULTS.extend([r1, r2])

    # ---------------- unshard: scatter window columns to node rows
    out_full = np.zeros((n_heads, n_nodes, d), dtype=np.float32)
    for c in range(N_CORES):
        slab = r2.results[c]["out"]      # [4, G, 128, OPG] f16
        cm = prep["cmaps"][c].reshape(-1)
        arr = slab.transpose(0, 1, 3, 2).reshape(n_heads, G * OPG, d)
        valid = cm >= 0
        out_full[:, cm[valid], :] = arr[:, valid, :].astype(np.float32)
    return out_full


if __name__ == "__main__":
    pass
